# revision 15
# baseline (speedup 1.0000x reference)
"""AUCLoss Trainium2 kernel (8-core SPMD, data-parallel over the sample dim).

Decomposition (validated against the jax reference):
  Samples are pre-sorted by class on the host, so each class occupies a
  contiguous run of columns.  For ordered pairs (a, b), ta = target[a]:
    M_ab = (proj[a,ta] - proj[b,ta]) / (wn[ta] * dn_ab)
    q_ab = min(r_ab, 1) * scb_b * (projn[b,ta] - u_a) * [tb != ta]
  with r = rsqrt(d2 + eps), scb_b = 1/sqrt(counts[tb]).  Then per class i
    W1_i = -sum_{a in i} cb_a * sum_j scb_j * (sum_{b in class j} q_ab)
    W2_i =  sum_{a in i} cb_a * sum_b q_ab^2
  and loss1/loss2 follow in O(C) on the host.

  On-device per 128-row chunk x 1024-col group (bf16 matmuls, f16 elsewhere):
    d2   = sq_b - 2 G     (PE: K=128 bf16 G-matmul;  the K=2 split-sq rows
           ride at tile_position (32,0) CONCURRENT with the K=31 tp-matmul
           at rows 0-30 -- different PE row-groups execute in parallel, so
           the sq matmuls are ~free)
    r    = Abs_reciprocal_sqrt(d2 + sq_a + EPS_D)   (ACT, one pass; |x|
           absorbs rounding-negative d2, EPS_D keeps the diagonal finite)
    q    = min(r,1) * tp  (DVE stt, per class-column segment, each
           segment's row-sum accumulated -> W1 segment columns)
    w2   = sum q^2        (ACT Square-accum or DVE stt q*q-accum,
           alternating per group to balance the two engines)
  Row stats [128, nseg+4] per chunk are DMA'd out; the final per-class
  scatter and scalar assembly run on the host.
"""

import numpy as np
from contextlib import ExitStack

import concourse.bass as bass
import concourse.tile as tile
from concourse import bacc, mybir
from concourse.bass_utils import run_bass_kernel_spmd

N = 4096
D = 128
C = 10
NCORES = 8
ROWS = N // NCORES          # 512 rows per core
CHUNKS = ROWS // 128        # 4
GW = 512                    # col group width (1 PSUM bank per tile)
NG = N // GW                # 8
K3 = C * 3 + 1              # 31 logical rows of the tp-matmul (single f16)
SQROW = 32                  # sq hi/lo rows at partitions 32-33, tile (32,0)
KP = 34                     # partitions of the packed tp/sq tensor
RCLAMP = 1.0
EPS_D = 0.01   # rsqrt bias: diagonal d2 cancels to ~0 +- 3e-5 fp noise, so
               # +0.01 keeps r_diag <= ~10 (then min-clamped); off-diagonal
               # d2 >= ~120 so the bias error is <= 4e-5 relative
RSQRT_WIDE = False   # single 1024-wide ACT rsqrt vs 2x512 (PSUM bank reads)

f32 = mybir.dt.float32
f16 = mybir.dt.float16
bf16 = mybir.dt.bfloat16
NPBF16 = mybir.dt.np(bf16)

_CACHE = {}

# exposed for test.py
LAST_RESULTS = None


def _build_nc(seg_layout):
    """seg_layout: tuple per group of ((start, end, segidx), ...) column
    segments (class runs clipped to the group)."""
    nseg = 1 + max(s[2] for g in seg_layout for s in g)
    ocols = nseg + NG                       # w1 segments | w2 per group
    nc = bacc.Bacc("TRN2", target_bir_lowering=False, debug=False)

    rhs_g = nc.dram_tensor("rhs_g", [D, N], f16, kind="ExternalInput")
    lhs_g = nc.dram_tensor("lhs_g", [D, ROWS], f16, kind="ExternalInput")
    rhs3a = nc.dram_tensor("rhs3a", [KP, N], f16, kind="ExternalInput")
    lhs3 = nc.dram_tensor("lhs3", [KP, ROWS], f16, kind="ExternalInput")
    biases = nc.dram_tensor("biases", [128, CHUNKS], f32, kind="ExternalInput")
    out_d = nc.dram_tensor("out", [128, ocols * CHUNKS], f32, kind="ExternalOutput")

    with ExitStack() as ctx:
        tc = ctx.enter_context(tile.TileContext(nc))
        singles = ctx.enter_context(tc.tile_pool(name="singles", bufs=1))
        pd2 = ctx.enter_context(tc.tile_pool(name="pd2", bufs=4, space="PSUM"))
        ptp = ctx.enter_context(tc.tile_pool(name="ptp", bufs=4, space="PSUM"))
        work = ctx.enter_context(tc.tile_pool(name="work", bufs=4))
        stats = ctx.enter_context(tc.tile_pool(name="stats", bufs=4))

        # ---- load inputs ----
        # group-0 dependencies first, strictly in consumption order; strips
        # split between the sync and gpsimd queues so transfers overlap.
        # lhs3/rhs3a pack: rows 0-30 = tp weights, rows 32-33 = ones / sq
        # hi-lo rows (consumed at PE tile_position (32,0)).
        sb_lhsg = singles.tile([D, ROWS], f16)
        sb_l3 = singles.tile([KP, ROWS], f16)
        sb_bias = singles.tile([128, CHUNKS], f32)
        sb_rhsg = [
            singles.tile([D, 1024], f16, tag=f"rhsg{s}", name=f"rhsg{s}")
            for s in range(N // 1024)
        ]
        sb_r3a = [
            singles.tile([KP, 1024], f16, tag=f"r3a{s}", name=f"r3a{s}")
            for s in range(N // 1024)
        ]
        nc.sync.dma_start(out=sb_lhsg, in_=lhs_g[:, :])
        nc.sync.dma_start(out=sb_rhsg[0], in_=rhs_g[:, 0:1024])
        nc.gpsimd.dma_start(out=sb_l3, in_=lhs3[:, :])
        nc.gpsimd.dma_start(out=sb_r3a[0], in_=rhs3a[:, 0:1024])
        nc.gpsimd.dma_start(out=sb_bias, in_=biases[:, :])
        for s in range(1, N // 1024):
            ssl = slice(s * 1024, (s + 1) * 1024)
            nc.sync.dma_start(out=sb_rhsg[s], in_=rhs_g[:, ssl])
            nc.gpsimd.dma_start(out=sb_r3a[s], in_=rhs3a[:, ssl])

        for c in range(CHUNKS):
            csl = slice(c * 128, (c + 1) * 128)
            st = stats.tile([128, ocols], f32, tag="st")
            for gb in range(0, NG, 4):
                batch = range(gb, gb + 4)
                d2s, tps = {}, {}
                for g in batch:
                    d2s[g] = pd2.tile([128, GW], f32, tag="d2", name=f"d2_{c}_{g}")
                    tps[g] = ptp.tile([128, GW], f32, tag="tp", name=f"tp_{c}_{g}")
                # 4 G-matmuls back-to-back: one LDWEIGHTS, pipelined issue
                for g in batch:
                    hs = slice((g % 2) * 512, (g % 2) * 512 + 512)
                    nc.tensor.matmul(
                        d2s[g], lhsT=sb_lhsg[:, csl], rhs=sb_rhsg[g // 2][:, hs],
                        start=True, stop=False, skip_group_check=True,
                    )
                # tp (K=31, PE rows 0-30, tp bank) and sq (K=2, rows 32-33,
                # d2 bank) differ in both row-group and PSUM bank: each
                # adjacent pair executes concurrently, sq is ~free.  One
                # LDWEIGHTS pair covers all 4 groups.
                for g in batch:
                    hs = slice((g % 2) * 512, (g % 2) * 512 + 512)
                    nc.tensor.matmul(
                        tps[g], lhsT=sb_l3[0:K3, csl],
                        rhs=sb_r3a[g // 2][0:K3, hs], start=True, stop=True,
                    )
                    nc.tensor.matmul(
                        d2s[g], lhsT=sb_l3[SQROW:SQROW + 2, csl],
                        rhs=sb_r3a[g // 2][SQROW:SQROW + 2, hs], start=False,
                        stop=True, tile_position=(SQROW, 0),
                        skip_group_check=True,
                    )
                for g in batch:
                    d2t, tp = d2s[g], tps[g]
                    r = work.tile([128, GW], f16, tag="r")
                    nc.scalar.activation(
                        r, d2t,
                        mybir.ActivationFunctionType.Abs_reciprocal_sqrt,
                        bias=sb_bias[:, c:c + 1], scale=1.0,
                    )
                    q = work.tile([128, GW], f16, tag="q")
                    for (s, e, sidx) in seg_layout[g]:
                        ls = slice(s - g * GW, e - g * GW)
                        nc.vector.scalar_tensor_tensor(
                            out=q[:, ls], in0=r[:, ls], scalar=RCLAMP,
                            in1=tp[:, ls], op0=mybir.AluOpType.min,
                            op1=mybir.AluOpType.mult,
                            accum_out=st[:, sidx:sidx + 1],
                        )
                    junk = work.tile([128, GW], f16, tag="junk")
                    w2col = st[:, nseg + g:nseg + g + 1]
                    if (c * NG + g) % 8 in (0, 2, 4):
                        nc.scalar.activation(
                            junk, q, mybir.ActivationFunctionType.Square,
                            accum_out=w2col,
                        )
                    else:
                        nc.vector.scalar_tensor_tensor(
                            out=junk, in0=q, scalar=1.0, in1=q,
                            op0=mybir.AluOpType.mult, op1=mybir.AluOpType.mult,
                            accum_out=w2col,
                        )
            nc.sync.dma_start(out=out_d[:, c * ocols:(c + 1) * ocols], in_=st)

    nc.compile()
    return nc, nseg, ocols


def _bf_split(x):
    """x (float64/32) -> (hi, lo) bf16 arrays with hi + lo ~= x."""
    x32 = np.asarray(x, np.float32)
    hi = x32.astype(NPBF16)
    lo = (x32 - hi.astype(np.float32)).astype(NPBF16)
    return hi, lo


def _segments(counts):
    """Class runs in sorted-column order, clipped to GW-wide groups.
    Returns (seg_layout, seg_class): per-group tuples of (s, e, segidx)
    and the class id of each segidx."""
    bounds = np.concatenate([[0], np.cumsum(counts)])
    seg_layout = []
    seg_class = []
    sidx = 0
    for g in range(NG):
        g0, g1 = g * GW, (g + 1) * GW
        segs = []
        for j in range(C):
            s, e = max(bounds[j], g0), min(bounds[j + 1], g1)
            if s < e:
                segs.append((int(s), int(e), sidx))
                seg_class.append(j)
                sidx += 1
        seg_layout.append(tuple(segs))
    return tuple(seg_layout), np.asarray(seg_class)


def _prep_inputs(pred, target, W):
    pred = np.asarray(pred, dtype=np.float32)
    target = np.asarray(target).astype(np.int64)
    W = np.asarray(W, dtype=np.float32)

    perm = np.argsort(target, kind="stable")
    pred = pred[perm]
    target = target[perm]

    p64 = pred.astype(np.float64)
    sq = (p64 * p64).sum(1)                                   # [N]
    wn = np.maximum(np.sqrt((W.astype(np.float64) ** 2).sum(1)), 1e-8)
    projn = (p64 @ W.T.astype(np.float64)) / wn[None, :]      # [N, C]
    counts = np.bincount(target, minlength=C)
    scb = 1.0 / np.sqrt(np.maximum(counts, 1))                # [C]
    colw = scb[target]                                        # [N]
    u = projn[np.arange(N), target]                           # [N]
    onehot = (target[:, None] == np.arange(C)[None, :]).astype(np.float64)

    NPF16 = np.float16
    gh = (-2.0 * pred.T).astype(NPF16)                        # f16 for G
    sqh = sq.astype(np.float32).astype(NPF16)
    sql = (sq - sqh.astype(np.float64)).astype(NPF16)

    m = np.empty((K3, N), dtype=np.float64)
    m[0:C] = colw[None, :] * projn.T
    m[C] = -colw
    m[C + 1:2 * C + 1] = -(colw[None, :] * (onehot * projn).T)
    m[2 * C + 1:3 * C + 1] = colw[None, :] * onehot.T
    r3a_full = np.zeros((KP, N), dtype=NPF16)
    r3a_full[0:K3] = m.astype(NPF16)
    r3a_full[SQROW] = sqh
    r3a_full[SQROW + 1] = sql

    l3_64 = np.empty((K3, N), dtype=np.float64)
    l3_64[0:C] = onehot.T
    l3_64[C] = u
    l3_64[C + 1:2 * C + 1] = onehot.T
    l3_64[2 * C + 1:3 * C + 1] = (u[:, None] * onehot).T
    l3_full = np.zeros((KP, N), dtype=NPF16)
    l3_full[0:K3] = l3_64.astype(NPF16)
    l3_full[SQROW:SQROW + 2] = 1.0

    in_maps = []
    for k in range(NCORES):
        rs = slice(k * ROWS, (k + 1) * ROWS)
        sq_own = sq[rs].reshape(CHUNKS, 128).T                # [128, CHUNKS]
        in_maps.append(
            {
                "rhs_g": np.ascontiguousarray(gh),
                "lhs_g": np.ascontiguousarray(pred.T[:, rs].astype(np.float16)),
                "rhs3a": np.ascontiguousarray(r3a_full),
                "lhs3": np.ascontiguousarray(l3_full[:, rs]),
                "biases": (sq_own + EPS_D).astype(np.float32),
            }
        )
    aux = {"counts": counts, "target": target, "scb": scb}
    return in_maps, aux


def _finish(per_core_out, aux, nseg, ocols, seg_class):
    counts, target, scb = aux["counts"], aux["target"], aux["scb"]
    cb = 1.0 / np.maximum(counts, 1)
    cb_a = cb[target]                                         # [N]
    mu = scb[seg_class]                                       # per-segment factor
    S = np.zeros((C, 2), dtype=np.float64)
    for k, o in enumerate(per_core_out):
        o = o.astype(np.float64)
        for c in range(CHUNKS):
            rs = slice(k * ROWS + c * 128, k * ROWS + (c + 1) * 128)
            oc = o[:, c * ocols:(c + 1) * ocols]
            inner1 = oc[:, :nseg] @ mu                        # [128]
            w2row = oc[:, nseg:nseg + NG].sum(1)              # [128]
            wrow = cb_a[rs]                                   # [128]
            tcls = target[rs]
            np.add.at(S[:, 0], tcls, wrow * inner1)
            np.add.at(S[:, 1], tcls, wrow * w2row)
    exist = float((counts > 0).sum())
    valid = counts > 0
    W1 = -S[:, 0]
    W2 = S[:, 1]
    W0 = exist - 1.0
    denom = exist - 1.0
    l1 = (W0 - 2.0 * W1 + W2) / denom
    mmn = W1 / denom
    mv = (W2 - 2.0 * mmn * W1 + mmn * mmn * W0) / denom
    safe_mm = np.where(mmn == 0.0, 1.0, mmn)
    loss1 = float(np.where(valid, l1, 0.0).sum() / exist)
    loss2 = float(np.where(valid, np.abs(mv / safe_mm), 0.0).sum() / exist)
    return (
        np.asarray(loss1, dtype=np.float32),
        np.asarray(loss2, dtype=np.float32),
    )


def kernel(pred, target, W):
    global LAST_RESULTS
    in_maps, aux = _prep_inputs(pred, target, W)
    seg_layout, seg_class = _segments(aux["counts"])
    if seg_layout not in _CACHE:
        _CACHE[seg_layout] = _build_nc(seg_layout)
    nc, nseg, ocols = _CACHE[seg_layout]
    res = run_bass_kernel_spmd(nc, in_maps, list(range(NCORES)))
    LAST_RESULTS = res
    per_core = [res.results[k]["out"] for k in range(NCORES)]
    return _finish(per_core, aux, nseg, ocols, seg_class)


# revision 16
# speedup vs baseline: 1.0066x; 1.0066x over previous
"""AUCLoss Trainium2 kernel (8-core SPMD, data-parallel over the sample dim).

Decomposition (validated against the jax reference):
  Samples are pre-sorted by class on the host, so each class occupies a
  contiguous run of columns.  For ordered pairs (a, b), ta = target[a]:
    M_ab = (proj[a,ta] - proj[b,ta]) / (wn[ta] * dn_ab)
    q_ab = min(r_ab, 1) * scb_b * (projn[b,ta] - u_a) * [tb != ta]
  with r = rsqrt(d2 + eps), scb_b = 1/sqrt(counts[tb]).  Then per class i
    W1_i = -sum_{a in i} cb_a * sum_j scb_j * (sum_{b in class j} q_ab)
    W2_i =  sum_{a in i} cb_a * sum_b q_ab^2
  and loss1/loss2 follow in O(C) on the host.

  On-device per 128-row chunk x 1024-col group (bf16 matmuls, f16 elsewhere):
    d2   = sq_b - 2 G     (PE: K=128 bf16 G-matmul;  the K=2 split-sq rows
           ride at tile_position (32,0) CONCURRENT with the K=31 tp-matmul
           at rows 0-30 -- different PE row-groups execute in parallel, so
           the sq matmuls are ~free)
    r    = Abs_reciprocal_sqrt(d2 + sq_a + EPS_D)   (ACT, one pass; |x|
           absorbs rounding-negative d2, EPS_D keeps the diagonal finite)
    q    = min(r,1) * tp  (DVE stt, per class-column segment, each
           segment's row-sum accumulated -> W1 segment columns)
    w2   = sum q^2        (ACT Square-accum or DVE stt q*q-accum,
           alternating per group to balance the two engines)
  Row stats [128, nseg+4] per chunk are DMA'd out; the final per-class
  scatter and scalar assembly run on the host.
"""

import numpy as np
from contextlib import ExitStack

import concourse.bass as bass
import concourse.tile as tile
from concourse import bacc, mybir
from concourse.bass_utils import run_bass_kernel_spmd

N = 4096
D = 128
C = 10
NCORES = 8
ROWS = N // NCORES          # 512 rows per core
CHUNKS = ROWS // 128        # 4
GW = 512                    # col group width (1 PSUM bank per tile)
NG = N // GW                # 8
K3 = C * 3 + 1              # 31 logical rows of the tp-matmul (single f16)
SQROW = 32                  # sq hi/lo rows at partitions 32-33, tile (32,0)
KP = 34                     # partitions of the packed tp/sq tensor
RCLAMP = 1.0
EPS_D = 0.01   # rsqrt bias: diagonal d2 cancels to ~0 +- 3e-5 fp noise, so
               # +0.01 keeps r_diag <= ~10 (then min-clamped); off-diagonal
               # d2 >= ~120 so the bias error is <= 4e-5 relative
RSQRT_WIDE = False   # single 1024-wide ACT rsqrt vs 2x512 (PSUM bank reads)

f32 = mybir.dt.float32
f16 = mybir.dt.float16
bf16 = mybir.dt.bfloat16
NPBF16 = mybir.dt.np(bf16)

_CACHE = {}

# exposed for test.py
LAST_RESULTS = None


def _build_nc(seg_layout):
    """seg_layout: tuple per group of ((start, end, segidx), ...) column
    segments (class runs clipped to the group)."""
    nseg = 1 + max(s[2] for g in seg_layout for s in g)
    ocols = nseg + NG                       # w1 segments | w2 per group
    nc = bacc.Bacc("TRN2", target_bir_lowering=False, debug=False)

    rhs_g = nc.dram_tensor("rhs_g", [D, N], f16, kind="ExternalInput")
    lhs_g = nc.dram_tensor("lhs_g", [D, ROWS], f16, kind="ExternalInput")
    rhs3a = nc.dram_tensor("rhs3a", [KP, N], f16, kind="ExternalInput")
    lhs3 = nc.dram_tensor("lhs3", [KP, ROWS], f16, kind="ExternalInput")
    biases = nc.dram_tensor("biases", [128, CHUNKS], f32, kind="ExternalInput")
    out_d = nc.dram_tensor("out", [128, ocols * CHUNKS], f32, kind="ExternalOutput")

    with ExitStack() as ctx:
        tc = ctx.enter_context(tile.TileContext(nc))
        singles = ctx.enter_context(tc.tile_pool(name="singles", bufs=1))
        pd2 = ctx.enter_context(tc.tile_pool(name="pd2", bufs=4, space="PSUM"))
        ptp = ctx.enter_context(tc.tile_pool(name="ptp", bufs=4, space="PSUM"))
        work = ctx.enter_context(tc.tile_pool(name="work", bufs=4))
        stats = ctx.enter_context(tc.tile_pool(name="stats", bufs=4))

        # ---- load inputs ----
        # group-0 dependencies first, strictly in consumption order; strips
        # split between the sync and gpsimd queues so transfers overlap.
        # lhs3/rhs3a pack: rows 0-30 = tp weights, rows 32-33 = ones / sq
        # hi-lo rows (consumed at PE tile_position (32,0)).
        sb_lhsg = singles.tile([D, ROWS], f16)
        sb_l3 = singles.tile([KP, ROWS], f16)
        sb_bias = singles.tile([128, CHUNKS], f32)
        sb_rhsg = [
            singles.tile([D, 1024], f16, tag=f"rhsg{s}", name=f"rhsg{s}")
            for s in range(N // 1024)
        ]
        sb_r3a = [
            singles.tile([KP, 1024], f16, tag=f"r3a{s}", name=f"r3a{s}")
            for s in range(N // 1024)
        ]
        nc.sync.dma_start(out=sb_lhsg, in_=lhs_g[:, :])
        nc.sync.dma_start(out=sb_rhsg[0], in_=rhs_g[:, 0:1024])
        nc.gpsimd.dma_start(out=sb_l3, in_=lhs3[:, :])
        nc.gpsimd.dma_start(out=sb_r3a[0], in_=rhs3a[:, 0:1024])
        nc.gpsimd.dma_start(out=sb_bias, in_=biases[:, :])
        for s in range(1, N // 1024):
            ssl = slice(s * 1024, (s + 1) * 1024)
            nc.sync.dma_start(out=sb_rhsg[s], in_=rhs_g[:, ssl])
            nc.gpsimd.dma_start(out=sb_r3a[s], in_=rhs3a[:, ssl])

        for c in range(CHUNKS):
            csl = slice(c * 128, (c + 1) * 128)
            st = stats.tile([128, ocols], f32, tag="st")
            for g in range(NG):
                d2t = pd2.tile([128, GW], f32, tag="d2", name=f"d2_{c}_{g}")
                tp = ptp.tile([128, GW], f32, tag="tp", name=f"tp_{c}_{g}")
                hs = slice((g % 2) * 512, (g % 2) * 512 + 512)
                nc.tensor.matmul(
                    d2t, lhsT=sb_lhsg[:, csl], rhs=sb_rhsg[g // 2][:, hs],
                    start=True, stop=False, skip_group_check=True,
                )
                # tp (K=31, PE rows 0-30, tp bank) and sq (K=2, rows 32-33,
                # d2 bank) differ in both row-group and PSUM bank, so the
                # adjacent pair executes concurrently -- sq is ~free.
                nc.tensor.matmul(
                    tp, lhsT=sb_l3[0:K3, csl],
                    rhs=sb_r3a[g // 2][0:K3, hs], start=True, stop=True,
                )
                nc.tensor.matmul(
                    d2t, lhsT=sb_l3[SQROW:SQROW + 2, csl],
                    rhs=sb_r3a[g // 2][SQROW:SQROW + 2, hs], start=False,
                    stop=True, tile_position=(SQROW, 0), skip_group_check=True,
                )
                r = work.tile([128, GW], f16, tag="r")
                nc.scalar.activation(
                    r, d2t,
                    mybir.ActivationFunctionType.Abs_reciprocal_sqrt,
                    bias=sb_bias[:, c:c + 1], scale=1.0,
                )
                q = work.tile([128, GW], f16, tag="q")
                for (s, e, sidx) in seg_layout[g]:
                    ls = slice(s - g * GW, e - g * GW)
                    nc.vector.scalar_tensor_tensor(
                        out=q[:, ls], in0=r[:, ls], scalar=RCLAMP,
                        in1=tp[:, ls], op0=mybir.AluOpType.min,
                        op1=mybir.AluOpType.mult,
                        accum_out=st[:, sidx:sidx + 1],
                    )
                junk = work.tile([128, GW], f16, tag="junk")
                w2col = st[:, nseg + g:nseg + g + 1]
                if (c * NG + g) % 8 in (0, 2, 4):
                    nc.scalar.activation(
                        junk, q, mybir.ActivationFunctionType.Square,
                        accum_out=w2col,
                    )
                else:
                    nc.vector.scalar_tensor_tensor(
                        out=junk, in0=q, scalar=1.0, in1=q,
                        op0=mybir.AluOpType.mult, op1=mybir.AluOpType.mult,
                        accum_out=w2col,
                    )
            nc.sync.dma_start(out=out_d[:, c * ocols:(c + 1) * ocols], in_=st)

    nc.compile()
    return nc, nseg, ocols


def _bf_split(x):
    """x (float64/32) -> (hi, lo) bf16 arrays with hi + lo ~= x."""
    x32 = np.asarray(x, np.float32)
    hi = x32.astype(NPBF16)
    lo = (x32 - hi.astype(np.float32)).astype(NPBF16)
    return hi, lo


def _segments(counts):
    """Class runs in sorted-column order, clipped to GW-wide groups.
    Returns (seg_layout, seg_class): per-group tuples of (s, e, segidx)
    and the class id of each segidx."""
    bounds = np.concatenate([[0], np.cumsum(counts)])
    seg_layout = []
    seg_class = []
    sidx = 0
    for g in range(NG):
        g0, g1 = g * GW, (g + 1) * GW
        segs = []
        for j in range(C):
            s, e = max(bounds[j], g0), min(bounds[j + 1], g1)
            if s < e:
                segs.append((int(s), int(e), sidx))
                seg_class.append(j)
                sidx += 1
        seg_layout.append(tuple(segs))
    return tuple(seg_layout), np.asarray(seg_class)


def _prep_inputs(pred, target, W):
    pred = np.asarray(pred, dtype=np.float32)
    target = np.asarray(target).astype(np.int64)
    W = np.asarray(W, dtype=np.float32)

    perm = np.argsort(target, kind="stable")
    pred = pred[perm]
    target = target[perm]

    p64 = pred.astype(np.float64)
    sq = (p64 * p64).sum(1)                                   # [N]
    wn = np.maximum(np.sqrt((W.astype(np.float64) ** 2).sum(1)), 1e-8)
    projn = (p64 @ W.T.astype(np.float64)) / wn[None, :]      # [N, C]
    counts = np.bincount(target, minlength=C)
    scb = 1.0 / np.sqrt(np.maximum(counts, 1))                # [C]
    colw = scb[target]                                        # [N]
    u = projn[np.arange(N), target]                           # [N]
    onehot = (target[:, None] == np.arange(C)[None, :]).astype(np.float64)

    NPF16 = np.float16
    gh = (-2.0 * pred.T).astype(NPF16)                        # f16 for G
    sqh = sq.astype(np.float32).astype(NPF16)
    sql = (sq - sqh.astype(np.float64)).astype(NPF16)

    m = np.empty((K3, N), dtype=np.float64)
    m[0:C] = colw[None, :] * projn.T
    m[C] = -colw
    m[C + 1:2 * C + 1] = -(colw[None, :] * (onehot * projn).T)
    m[2 * C + 1:3 * C + 1] = colw[None, :] * onehot.T
    r3a_full = np.zeros((KP, N), dtype=NPF16)
    r3a_full[0:K3] = m.astype(NPF16)
    r3a_full[SQROW] = sqh
    r3a_full[SQROW + 1] = sql

    l3_64 = np.empty((K3, N), dtype=np.float64)
    l3_64[0:C] = onehot.T
    l3_64[C] = u
    l3_64[C + 1:2 * C + 1] = onehot.T
    l3_64[2 * C + 1:3 * C + 1] = (u[:, None] * onehot).T
    l3_full = np.zeros((KP, N), dtype=NPF16)
    l3_full[0:K3] = l3_64.astype(NPF16)
    l3_full[SQROW:SQROW + 2] = 1.0

    in_maps = []
    for k in range(NCORES):
        rs = slice(k * ROWS, (k + 1) * ROWS)
        sq_own = sq[rs].reshape(CHUNKS, 128).T                # [128, CHUNKS]
        in_maps.append(
            {
                "rhs_g": np.ascontiguousarray(gh),
                "lhs_g": np.ascontiguousarray(pred.T[:, rs].astype(np.float16)),
                "rhs3a": np.ascontiguousarray(r3a_full),
                "lhs3": np.ascontiguousarray(l3_full[:, rs]),
                "biases": (sq_own + EPS_D).astype(np.float32),
            }
        )
    aux = {"counts": counts, "target": target, "scb": scb}
    return in_maps, aux


def _finish(per_core_out, aux, nseg, ocols, seg_class):
    counts, target, scb = aux["counts"], aux["target"], aux["scb"]
    cb = 1.0 / np.maximum(counts, 1)
    cb_a = cb[target]                                         # [N]
    mu = scb[seg_class]                                       # per-segment factor
    S = np.zeros((C, 2), dtype=np.float64)
    for k, o in enumerate(per_core_out):
        o = o.astype(np.float64)
        for c in range(CHUNKS):
            rs = slice(k * ROWS + c * 128, k * ROWS + (c + 1) * 128)
            oc = o[:, c * ocols:(c + 1) * ocols]
            inner1 = oc[:, :nseg] @ mu                        # [128]
            w2row = oc[:, nseg:nseg + NG].sum(1)              # [128]
            wrow = cb_a[rs]                                   # [128]
            tcls = target[rs]
            np.add.at(S[:, 0], tcls, wrow * inner1)
            np.add.at(S[:, 1], tcls, wrow * w2row)
    exist = float((counts > 0).sum())
    valid = counts > 0
    W1 = -S[:, 0]
    W2 = S[:, 1]
    W0 = exist - 1.0
    denom = exist - 1.0
    l1 = (W0 - 2.0 * W1 + W2) / denom
    mmn = W1 / denom
    mv = (W2 - 2.0 * mmn * W1 + mmn * mmn * W0) / denom
    safe_mm = np.where(mmn == 0.0, 1.0, mmn)
    loss1 = float(np.where(valid, l1, 0.0).sum() / exist)
    loss2 = float(np.where(valid, np.abs(mv / safe_mm), 0.0).sum() / exist)
    return (
        np.asarray(loss1, dtype=np.float32),
        np.asarray(loss2, dtype=np.float32),
    )


def kernel(pred, target, W):
    global LAST_RESULTS
    in_maps, aux = _prep_inputs(pred, target, W)
    seg_layout, seg_class = _segments(aux["counts"])
    if seg_layout not in _CACHE:
        _CACHE[seg_layout] = _build_nc(seg_layout)
    nc, nseg, ocols = _CACHE[seg_layout]
    res = run_bass_kernel_spmd(nc, in_maps, list(range(NCORES)))
    LAST_RESULTS = res
    per_core = [res.results[k]["out"] for k in range(NCORES)]
    return _finish(per_core, aux, nseg, ocols, seg_class)
